# revision 35
# baseline (speedup 1.0000x reference)
"""Trainium2 Bass kernel for nn_ContrastLoss (contrastive PSD loss).

Math notes (validated against the jax reference):
  * The band (rfft bins 92..568 of a 4096-point DFT) excludes DC, so the
    mean subtraction in the reference is a no-op for the band PSD.
  * diag(D) == 0 for the pairwise-MSE matrix, and every _compare() term
    reduces to rank-1 statistics of the normalized PSD matrices:
        sum_ij D_ij * F = M*SSQ_a + N*SSQ_b - 2 * cs_a . cs_b
    with SSQ = sum of squared entries and cs = column sums.  So the NxN
    Gram matrix is never materialized; the device only produces per-core
    column sums and per-row (sum, sum-of-squares) statistics.
  * Even/odd frequency split: for even k, X_k = DFT_2048(x0+x1)[k/2-ish];
    for odd k, X_k = sum_n (x0-x1)_n W_4096^{nk} (n < 2048).  This halves
    the matmul contract length.
  * fp8(e4m3) quantization of both crops and the DFT matrices keeps the
    final loss within ~7e-4 relative (validated numerically on the host);
    tolerance is 2e-2.  PE runs DoubleRow fp8 matmuls: 256-deep contract
    per instruction, halving instruction count vs 128-deep.

Device work per core (1024 crops of the 8192 total, 3 passes of 3/3/2
row blocks):
  crops [128 n-part][half][chunk-pair][ktile][crop]  (fp8)
  x  W [128 n-part][chunk-pair][ktile][480 = cos|sin band bins] (fp8)
  -> PSUM [128 crops, 478] per block/half (fp32), ACT Square -> sq,
  -> DVE add (p = C^2+S^2 halves, bf16) + reduce -> row sum rs,
  -> ACT Square accum -> row sum-of-squares q, DVE reciprocal,
  -> PE colsum matmul with lhsT = 1/rs (bf16) -> cs [1, 478].
Host combines the 8 cores' (cs, rs, q) in float64.

Timing notes: warm-up matmuls hold the PE HAM activity window busy
through the DMA-bound startup so the clock runs at 2.4 GHz for the real
matmuls (DoubleRow warm pitch 202 ns = 478 cols at 1 col/cycle, the
PSUM-write limit).  DMA issue costs ~700ns per descriptor on an engine
queue, so transfers are batched coarsely, W on the gpsimd ring and
crops on the sync ring.
"""

import numpy as np
import ml_dtypes

# Problem constants (hardcoded; kernel.py must be self-contained)
B, C, T = 2, 64, 32768
L = 4096
HALF = L // 2                  # 2048
K_CROPS = 32
N_ROWS = C * K_CROPS           # 2048 rows per PSD matrix
N_CORES = 8
ROWS_PER_CORE = N_ROWS * 4 // N_CORES   # 1024
NB = ROWS_PER_CORE // 128      # 8 row blocks per core
NP2 = 8                        # chunk-pairs (contract 256 each)
K_EVEN = np.arange(92, 569, 2)  # 239 even band bins
K_ODD = np.arange(93, 568, 2)   # 238 odd band bins
B2 = 239                       # padded bins per half (odd half: 238 + 1 zero)
CW = 2 * B2 + 2                # 480: padded W col width (C|S|2 zero pads)
FW = 2 * B2                    # 478: live cols per half tile
F = 477                        # real band bins (239 + 238)

_PASSES = [[0, 1, 2], [3, 4, 5], [6, 7]]

_NC = None
_W_CACHE = None


def _band_tables():
    """fp8 DFT matrices w_e, w_d laid out [128 part, pair, ktile, 480]."""
    global _W_CACHE
    if _W_CACHE is not None:
        return _W_CACHE
    f8 = ml_dtypes.float8_e4m3

    def mk(ks):
        n = np.arange(HALF, dtype=np.float64)[:, None]
        ph = 2.0 * np.pi * n * ks[None, :].astype(np.float64) / 4096.0
        w = np.zeros((HALF, CW), dtype=np.float32)
        w[:, 0:len(ks)] = np.cos(ph)
        w[:, B2:B2 + len(ks)] = np.sin(ph)
        # n = 256*pair + 128*ktile + part
        w = w.reshape(NP2, 2, 128, CW).transpose(2, 0, 1, 3)
        return np.ascontiguousarray(w).astype(f8)

    _W_CACHE = (mk(K_EVEN), mk(K_ODD))
    return _W_CACHE


def _build_module():
    global _NC
    if _NC is not None:
        return _NC
    import concourse.bacc as bacc
    import concourse.bass as bass
    import concourse.tile as tile
    from concourse import mybir

    f32 = mybir.dt.float32
    bf16 = mybir.dt.bfloat16
    f8 = mybir.dt.float8e4
    AF = mybir.ActivationFunctionType
    ALU = mybir.AluOpType
    DR = mybir.MatmulPerfMode.DoubleRow

    nc = bacc.Bacc("TRN2", target_bir_lowering=False, debug=False,
                   num_devices=N_CORES)

    w_e = nc.dram_tensor("w_e", [128, NP2, 2, CW], f8, kind="ExternalInput")
    w_d = nc.dram_tensor("w_d", [128, NP2, 2, CW], f8, kind="ExternalInput")
    crops_d = [
        nc.dram_tensor(f"crops_p{p}", [128, 2, NP2, 2, 128 * len(blks)], f8,
                       kind="ExternalInput")
        for p, blks in enumerate(_PASSES)
    ]
    out_cs = nc.dram_tensor("out_cs", [1, FW], f32, kind="ExternalOutput")
    # rs in cols 0..7, q in cols 8..15 (per row block)
    out_rq = nc.dram_tensor("out_rq", [128, 2 * NB], f32,
                            kind="ExternalOutput")

    with tile.TileContext(nc) as tc:
        with (
            tc.tile_pool(name="wp", bufs=1) as wp,
            tc.tile_pool(name="cp", bufs=2) as cp,
            tc.tile_pool(name="sq", bufs=2) as sqp,
            tc.tile_pool(name="pp", bufs=2) as ppool,
            tc.tile_pool(name="qq", bufs=2) as qpool,
            tc.tile_pool(name="sm", bufs=3) as sm,
            tc.tile_pool(name="outp", bufs=1) as outp,
            tc.tile_pool(name="pse", bufs=1, space=bass.MemorySpace.PSUM) as pse,
            tc.tile_pool(name="psd", bufs=1, space=bass.MemorySpace.PSUM) as psd,
            tc.tile_pool(name="pcs", bufs=1, space=bass.MemorySpace.PSUM) as pcs,
            tc.tile_pool(name="psw", bufs=1, space=bass.MemorySpace.PSUM) as psw,
        ):
            we_t = wp.tile([128, NP2, 2, CW], f8)
            wd_t = wp.tile([128, NP2, 2, CW], f8)
            rq_t = outp.tile([128, 2 * NB], f32)
            warm_sb = outp.tile([128, 256], bf16)
            cs_psum = pcs.tile([1, FW], f32)
            warm_ps = psw.tile([1, 256], f32)

            # HAM pre-warm: keep the PE activity window busy from the
            # prologue until the first crop DMA lands, so the clock
            # un-throttles (1.2 -> 2.4 GHz) early into the real matmuls.
            nc.gpsimd.memset(warm_sb, 0.0)
            for i in range(18):
                nc.tensor.matmul(warm_ps, warm_sb[:, 0:1], warm_sb,
                                 start=True, stop=True)

            pending = []   # deferred colsum matmuls (inv, p3, j, blk)

            def emit_cs(items):
                for c_inv, c_p3, c_j, c_blk in items:
                    nc.tensor.matmul(cs_psum, c_inv, c_p3[:, c_j],
                                     start=(c_blk == 0),
                                     stop=(c_blk == NB - 1))

            for p, blks in enumerate(_PASSES):
                nbp = len(blks)
                b0 = blks[0]
                cpass = cp.tile([128, 2, NP2, 2, 128 * nbp], f8, tag="cp")
                # DMAs in consumption order; W on the gpsimd queue, crops on
                # the sync queue so descriptor issue runs in parallel.
                if p == 0:
                    # fine-grained, consumption-ordered first wave: W chunks
                    # on the gpsimd ring, crops on the sync ring, so the
                    # chunk-pair-k matmuls start as soon as pair k lands.
                    # coarse splits: DMA issue costs ~700ns on the queue,
                    # so few big transfers beat many small ones.
                    for lo, hi in ((0, 1), (1, 3), (3, 5), (5, 8)):
                        nc.gpsimd.dma_start(out=we_t[:, lo:hi],
                                            in_=w_e[:, lo:hi])
                        nc.sync.dma_start(out=cpass[:, 0, lo:hi],
                                          in_=crops_d[p][:, 0, lo:hi])
                    for lo, hi in ((0, 2), (2, 5), (5, 8)):
                        nc.gpsimd.dma_start(out=wd_t[:, lo:hi],
                                            in_=w_d[:, lo:hi])
                        nc.sync.dma_start(out=cpass[:, 1, lo:hi],
                                          in_=crops_d[p][:, 1, lo:hi])
                else:
                    nc.sync.dma_start(out=cpass[:, 0], in_=crops_d[p][:, 0])
                    nc.sync.dma_start(out=cpass[:, 1], in_=crops_d[p][:, 1])

                e3 = pse.tile([128, 3, 512], f32, tag="e3")
                d3 = psd.tile([128, 3, 512], f32, tag="d3")

                if p < len(_PASSES) - 1:
                    # chunk-pair-outer matmuls (DoubleRow: contract 256).
                    for ch2 in range(NP2):
                        for j in range(nbp):
                            nc.tensor.matmul(
                                e3[:, j, 0:FW],
                                cpass[:, 0, ch2, :, 128 * j:128 * (j + 1)],
                                we_t[:, ch2, :, 0:FW],
                                start=(ch2 == 0), stop=(ch2 == NP2 - 1),
                                perf_mode=DR)
                    # previous pass' colsum matmuls (inv is ready by now)
                    if pending:
                        emit_cs(pending)
                        pending = []
                    for ch2 in range(NP2):
                        for j in range(nbp):
                            nc.tensor.matmul(
                                d3[:, j, 0:FW],
                                cpass[:, 1, ch2, :, 128 * j:128 * (j + 1)],
                                wd_t[:, ch2, :, 0:FW],
                                start=(ch2 == 0), stop=(ch2 == NP2 - 1),
                                perf_mode=DR)

                    # post-chain: sq = PSUM^2; p = C^2+S^2; rs = sum p;
                    # q = sum p^2; inv = 1/rs
                    sq3 = sqp.tile([128, nbp, 2, 2, B2], f32, tag="sq")
                    p3 = ppool.tile([128, nbp, 2, B2], bf16, tag="p")
                    nc.scalar.activation(
                        out=sq3[:, :, 0], func=AF.Square,
                        in_=e3[:, 0:nbp, 0:FW].rearrange(
                            "p j (c k) -> p j c k", c=2))
                    nc.scalar.activation(
                        out=sq3[:, :, 1], func=AF.Square,
                        in_=d3[:, 0:nbp, 0:FW].rearrange(
                            "p j (c k) -> p j c k", c=2))
                    with nc.allow_low_precision(reason="bf16 validated"):
                        for j, blk in enumerate(blks):
                            nc.vector.tensor_add(p3[:, j], sq3[:, j, :, 0],
                                                 sq3[:, j, :, 1])
                        for j, blk in enumerate(blks):
                            nc.vector.tensor_reduce(
                                out=rq_t[:, blk:blk + 1], in_=p3[:, j],
                                axis=mybir.AxisListType.XY, op=ALU.add)
                    inv3 = sm.tile([128, nbp], bf16, tag="inv",
                                   name=f"inv{p}")
                    with nc.allow_low_precision(reason="bf16 validated"):
                        nc.vector.reciprocal(inv3, rq_t[:, b0:b0 + nbp])
                    for j, blk in enumerate(blks):
                        psq = qpool.tile([128, FW], bf16, tag="psq",
                                         name=f"psq{blk}")
                        nc.scalar.activation(
                            out=psq, in_=p3[:, j], func=AF.Square,
                            accum_out=rq_t[:, NB + blk:NB + blk + 1])
                        pending.append((inv3[:, j:j + 1], p3, j, blk))
                else:
                    # final pass: block-outer matmuls + per-block post-chain
                    # so earlier blocks' stats chains overlap later blocks'
                    # matmuls (short kernel tail).  rs comes from the Square
                    # accumulators (rs_e + rs_d) to skip the 478-col reduce.
                    sq3 = sqp.tile([128, nbp, 2, 2, B2], f32, tag="sq")
                    p3 = ppool.tile([128, nbp, 2, B2], bf16, tag="p")
                    for j, blk in enumerate(blks):
                        for ch2 in range(NP2):
                            nc.tensor.matmul(
                                e3[:, j, 0:FW],
                                cpass[:, 0, ch2, :, 128 * j:128 * (j + 1)],
                                we_t[:, ch2, :, 0:FW],
                                start=(ch2 == 0), stop=(ch2 == NP2 - 1),
                                perf_mode=DR)
                        for ch2 in range(NP2):
                            nc.tensor.matmul(
                                d3[:, j, 0:FW],
                                cpass[:, 1, ch2, :, 128 * j:128 * (j + 1)],
                                wd_t[:, ch2, :, 0:FW],
                                start=(ch2 == 0), stop=(ch2 == NP2 - 1),
                                perf_mode=DR)
                        if pending:
                            emit_cs(pending)
                            pending = []
                        rse = sm.tile([128, 2], f32, tag="rse",
                                      name=f"rse{blk}")
                        nc.scalar.activation(
                            out=sq3[:, j, 0], func=AF.Square,
                            in_=e3[:, j, 0:FW].rearrange("p (c k) -> p c k",
                                                         c=2),
                            accum_out=rse[:, 0:1])
                        nc.scalar.activation(
                            out=sq3[:, j, 1], func=AF.Square,
                            in_=d3[:, j, 0:FW].rearrange("p (c k) -> p c k",
                                                         c=2),
                            accum_out=rse[:, 1:2])
                        inv1 = sm.tile([128, 1], bf16, tag="inv1",
                                       name=f"inv1_{blk}")
                        with nc.allow_low_precision(reason="bf16 validated"):
                            # e-half of p assembles during the d matmuls;
                            # only the d-half add sits on the kernel tail
                            nc.vector.tensor_add(p3[:, j, 0],
                                                 sq3[:, j, 0, 0],
                                                 sq3[:, j, 0, 1])
                            nc.gpsimd.tensor_add(p3[:, j, 1],
                                                 sq3[:, j, 1, 0],
                                                 sq3[:, j, 1, 1])
                            nc.vector.tensor_add(rq_t[:, blk:blk + 1],
                                                 rse[:, 0:1], rse[:, 1:2])
                            nc.vector.reciprocal(inv1,
                                                 rq_t[:, blk:blk + 1])
                        psq = qpool.tile([128, FW], bf16, tag="psq",
                                         name=f"psq{blk}")
                        nc.scalar.activation(
                            out=psq, in_=p3[:, j], func=AF.Square,
                            accum_out=rq_t[:, NB + blk:NB + blk + 1])
                        pending.append((inv1, p3, j, blk))

            emit_cs(pending)

            cs_sb = outp.tile([1, FW], f32)
            nc.vector.tensor_copy(cs_sb, cs_psum[:, 0:FW])
            nc.gpsimd.dma_start(out=out_rq[:], in_=rq_t)
            nc.sync.dma_start(out=out_cs[:], in_=cs_sb)

    nc.compile()
    _NC = nc
    return nc


def _host_prepare(model_output, GT_sig, offsets_st, offsets_t):
    """Build per-core in_maps: gather crops, e/d split, fp8 layout."""
    from numpy.lib.stride_tricks import sliding_window_view
    f8 = ml_dtypes.float8_e4m3
    w_e, w_d = _band_tables()
    mats = []   # 4 matrices' crop rows [2048, 4096] float32
    for b in range(B):
        offs = np.asarray(offsets_st[b], dtype=np.int64).reshape(-1)
        ch_idx = np.repeat(np.arange(C), K_CROPS)
        base = np.asarray(model_output[b], dtype=np.float32)
        win = sliding_window_view(base, L, axis=-1)  # [C, T-L+1, L]
        mats.append(win[ch_idx, offs])               # [2048, L]
    for b in range(B):
        offs = np.asarray(offsets_t[b], dtype=np.int64).reshape(-1)
        win = sliding_window_view(np.asarray(GT_sig[b], dtype=np.float32), L)
        mats.append(win[offs])

    in_maps = []
    for m in range(4):
        cr = mats[m]
        e = cr[:, :HALF] + cr[:, HALF:]
        d = cr[:, :HALF] - cr[:, HALF:]
        # [rows, 2, 2048] -> [rows, 2, pair, ktile, 128] -> [128, 2, pair, ktile, rows]
        x = np.stack([e, d], axis=1).reshape(-1, 2, NP2, 2, 128)
        for h in range(2):
            sl = x[h * ROWS_PER_CORE:(h + 1) * ROWS_PER_CORE]
            xr = np.ascontiguousarray(
                sl.transpose(4, 1, 2, 3, 0)).astype(f8)  # [128,2,8,2,1024]
            im = {"w_e": w_e, "w_d": w_d}
            c0 = 0
            for p, blks in enumerate(_PASSES):
                c1 = c0 + 128 * len(blks)
                im[f"crops_p{p}"] = np.ascontiguousarray(xr[..., c0:c1])
                c0 = c1
            in_maps.append(im)
    return in_maps


def _combine(results, label_flag):
    """results: list of 8 dicts with out_cs [1,478], out_rq [128,16]."""
    cs = np.zeros((4, FW), dtype=np.float64)
    ssq = np.zeros(4, dtype=np.float64)
    for m in range(4):
        for h in range(2):
            r = results[2 * m + h]
            cs[m] += np.asarray(r["out_cs"], dtype=np.float64)[0]
            rq = np.asarray(r["out_rq"], dtype=np.float64)
            rs = rq[:, 0:NB]
            q = rq[:, NB:2 * NB]
            ssq[m] += float(np.sum(q / (rs * rs)))

    N = float(N_ROWS)

    def cmp_excl(a):
        return (2.0 * N * ssq[a] - 2.0 * np.dot(cs[a], cs[a])) / F / (N * (N - 1.0))

    def cmp_full(a, b):
        return (N * ssq[a] + N * ssq[b] - 2.0 * np.dot(cs[a], cs[b])) / F / (N * N)

    lf = np.asarray(label_flag, dtype=np.float64).reshape(-1)
    lf_sum = lf[0] + lf[1]
    denom = 1.0 if lf_sum == 0 else lf_sum
    pos_loss = (cmp_excl(0) + cmp_excl(1)) / 2.0
    neg_loss = -cmp_full(0, 1)
    pos_GT = (lf[0] * cmp_full(0, 2) + lf[1] * cmp_full(1, 3)) / denom
    neg_GT = -(lf[0] * cmp_full(1, 2) + lf[1] * cmp_full(0, 3)) / denom
    if lf_sum == 0:
        pos_GT = 0.0
        neg_GT = 0.0
    loss = pos_loss + neg_loss + pos_GT + neg_GT
    return (np.float32(loss), np.float32(pos_loss), np.float32(neg_loss),
            np.float32(pos_GT), np.float32(neg_GT))


def run(inputs, trace=False):
    """Returns (outputs_tuple, BassKernelResults)."""
    from concourse import bass_utils
    nc = _build_module()
    in_maps = _host_prepare(
        inputs["model_output"], inputs["GT_sig"],
        inputs["offsets_st"], inputs["offsets_t"])
    res = bass_utils.run_bass_kernel_spmd(
        nc, in_maps, core_ids=list(range(N_CORES)), trace=trace)
    outs = _combine(res.results, inputs["label_flag"])
    return outs, res


def kernel(**inputs):
    outs, _ = run(inputs)
    return outs


# revision 36
# speedup vs baseline: 1.0181x; 1.0181x over previous
"""Trainium2 Bass kernel for nn_ContrastLoss (contrastive PSD loss).

Math notes (validated against the jax reference):
  * The band (rfft bins 92..568 of a 4096-point DFT) excludes DC, so the
    mean subtraction in the reference is a no-op for the band PSD.
  * diag(D) == 0 for the pairwise-MSE matrix, and every _compare() term
    reduces to rank-1 statistics of the normalized PSD matrices:
        sum_ij D_ij * F = M*SSQ_a + N*SSQ_b - 2 * cs_a . cs_b
    with SSQ = sum of squared entries and cs = column sums.  So the NxN
    Gram matrix is never materialized; the device only produces per-core
    column sums and per-row (sum, sum-of-squares) statistics.
  * Even/odd frequency split: for even k, X_k = DFT_2048(x0+x1)[k/2-ish];
    for odd k, X_k = sum_n (x0-x1)_n W_4096^{nk} (n < 2048).  This halves
    the matmul contract length.
  * fp8(e4m3) quantization of both crops and the DFT matrices keeps the
    final loss within ~7e-4 relative (validated numerically on the host);
    tolerance is 2e-2.  PE runs DoubleRow fp8 matmuls: 256-deep contract
    per instruction, halving instruction count vs 128-deep.

Device work per core (1024 crops of the 8192 total, 3 passes of 3/3/2
row blocks):
  crops [128 n-part][half][chunk-pair][ktile][crop]  (fp8)
  x  W [128 n-part][chunk-pair][ktile][480 = cos|sin band bins] (fp8)
  -> PSUM [128 crops, 478] per block/half (fp32), ACT Square -> sq,
  -> DVE add (p = C^2+S^2 halves, bf16) + reduce -> row sum rs,
  -> ACT Square accum -> row sum-of-squares q, DVE reciprocal,
  -> PE colsum matmul with lhsT = 1/rs (bf16) -> cs [1, 478].
Host combines the 8 cores' (cs, rs, q) in float64.

Timing notes: warm-up matmuls hold the PE HAM activity window busy
through the DMA-bound startup so the clock runs at 2.4 GHz for the real
matmuls (DoubleRow warm pitch 202 ns = 478 cols at 1 col/cycle, the
PSUM-write limit).  DMA issue costs ~700ns per descriptor on an engine
queue, so transfers are batched coarsely, W on the gpsimd ring and
crops on the sync ring.
"""

import numpy as np
import ml_dtypes

# Problem constants (hardcoded; kernel.py must be self-contained)
B, C, T = 2, 64, 32768
L = 4096
HALF = L // 2                  # 2048
K_CROPS = 32
N_ROWS = C * K_CROPS           # 2048 rows per PSD matrix
N_CORES = 8
ROWS_PER_CORE = N_ROWS * 4 // N_CORES   # 1024
NB = ROWS_PER_CORE // 128      # 8 row blocks per core
NP2 = 8                        # chunk-pairs (contract 256 each)
K_EVEN = np.arange(92, 569, 2)  # 239 even band bins
K_ODD = np.arange(93, 568, 2)   # 238 odd band bins
B2 = 239                       # padded bins per half (odd half: 238 + 1 zero)
CW = 2 * B2 + 2                # 480: padded W col width (C|S|2 zero pads)
FW = 2 * B2                    # 478: live cols per half tile
F = 477                        # real band bins (239 + 238)

_PASSES = [[0, 1, 2], [3, 4, 5], [6, 7]]

_NC = None
_W_CACHE = None


def _band_tables():
    """fp8 DFT matrices w_e, w_d laid out [128 part, pair, ktile, 480]."""
    global _W_CACHE
    if _W_CACHE is not None:
        return _W_CACHE
    f8 = ml_dtypes.float8_e4m3

    def mk(ks):
        n = np.arange(HALF, dtype=np.float64)[:, None]
        ph = 2.0 * np.pi * n * ks[None, :].astype(np.float64) / 4096.0
        w = np.zeros((HALF, CW), dtype=np.float32)
        w[:, 0:len(ks)] = np.cos(ph)
        w[:, B2:B2 + len(ks)] = np.sin(ph)
        # n = 256*pair + 128*ktile + part
        w = w.reshape(NP2, 2, 128, CW).transpose(2, 0, 1, 3)
        return np.ascontiguousarray(w).astype(f8)

    _W_CACHE = (mk(K_EVEN), mk(K_ODD))
    return _W_CACHE


def _build_module():
    global _NC
    if _NC is not None:
        return _NC
    import concourse.bacc as bacc
    import concourse.bass as bass
    import concourse.tile as tile
    from concourse import mybir

    f32 = mybir.dt.float32
    bf16 = mybir.dt.bfloat16
    f8 = mybir.dt.float8e4
    AF = mybir.ActivationFunctionType
    ALU = mybir.AluOpType
    DR = mybir.MatmulPerfMode.DoubleRow

    nc = bacc.Bacc("TRN2", target_bir_lowering=False, debug=False,
                   num_devices=N_CORES)

    w_e = nc.dram_tensor("w_e", [128, NP2, 2, CW], f8, kind="ExternalInput")
    w_d = nc.dram_tensor("w_d", [128, NP2, 2, CW], f8, kind="ExternalInput")
    crops_d = [
        nc.dram_tensor(f"crops_p{p}", [128, 2, NP2, 2, 128 * len(blks)], f8,
                       kind="ExternalInput")
        for p, blks in enumerate(_PASSES)
    ]
    out_cs = nc.dram_tensor("out_cs", [1, FW], f32, kind="ExternalOutput")
    # rs in cols 0..7, q in cols 8..15 (per row block)
    out_rq = nc.dram_tensor("out_rq", [128, 2 * NB], f32,
                            kind="ExternalOutput")

    with tile.TileContext(nc) as tc:
        with (
            tc.tile_pool(name="wp", bufs=1) as wp,
            tc.tile_pool(name="cp", bufs=2) as cp,
            tc.tile_pool(name="sq", bufs=2) as sqp,
            tc.tile_pool(name="pp", bufs=2) as ppool,
            tc.tile_pool(name="qq", bufs=2) as qpool,
            tc.tile_pool(name="sm", bufs=3) as sm,
            tc.tile_pool(name="outp", bufs=1) as outp,
            tc.tile_pool(name="pse", bufs=1, space=bass.MemorySpace.PSUM) as pse,
            tc.tile_pool(name="psd", bufs=1, space=bass.MemorySpace.PSUM) as psd,
            tc.tile_pool(name="pcs", bufs=1, space=bass.MemorySpace.PSUM) as pcs,
            tc.tile_pool(name="psw", bufs=1, space=bass.MemorySpace.PSUM) as psw,
        ):
            we_t = wp.tile([128, NP2, 2, CW], f8)
            wd_t = wp.tile([128, NP2, 2, CW], f8)
            rq_t = outp.tile([128, 2 * NB], f32)
            warm_sb = outp.tile([128, 256], bf16)
            cs_psum = pcs.tile([1, FW], f32)
            warm_ps = psw.tile([1, 256], f32)

            # HAM pre-warm: keep the PE activity window busy from the
            # prologue until the first crop DMA lands, so the clock
            # un-throttles (1.2 -> 2.4 GHz) early into the real matmuls.
            nc.gpsimd.memset(warm_sb, 0.0)
            for i in range(20):
                nc.tensor.matmul(warm_ps, warm_sb[:, 0:1], warm_sb,
                                 start=True, stop=True)

            pending = []   # deferred colsum matmuls (inv, p3, j, blk)

            def emit_cs(items):
                for c_inv, c_p3, c_j, c_blk in items:
                    nc.tensor.matmul(cs_psum, c_inv, c_p3[:, c_j],
                                     start=(c_blk == 0),
                                     stop=(c_blk == NB - 1))

            for p, blks in enumerate(_PASSES):
                nbp = len(blks)
                b0 = blks[0]
                cpass = cp.tile([128, 2, NP2, 2, 128 * nbp], f8, tag="cp")
                # DMAs in consumption order; W on the gpsimd queue, crops on
                # the sync queue so descriptor issue runs in parallel.
                if p == 0:
                    # fine-grained, consumption-ordered first wave: W chunks
                    # on the gpsimd ring, crops on the sync ring, so the
                    # chunk-pair-k matmuls start as soon as pair k lands.
                    # coarse splits: DMA issue costs ~700ns on the queue,
                    # so few big transfers beat many small ones.
                    for lo, hi in ((0, 1), (1, 3), (3, 5), (5, 8)):
                        nc.gpsimd.dma_start(out=we_t[:, lo:hi],
                                            in_=w_e[:, lo:hi])
                        nc.sync.dma_start(out=cpass[:, 0, lo:hi],
                                          in_=crops_d[p][:, 0, lo:hi])
                    for lo, hi in ((0, 2), (2, 5), (5, 8)):
                        nc.gpsimd.dma_start(out=wd_t[:, lo:hi],
                                            in_=w_d[:, lo:hi])
                        nc.sync.dma_start(out=cpass[:, 1, lo:hi],
                                          in_=crops_d[p][:, 1, lo:hi])
                else:
                    nc.sync.dma_start(out=cpass[:, 0], in_=crops_d[p][:, 0])
                    nc.sync.dma_start(out=cpass[:, 1], in_=crops_d[p][:, 1])

                e3 = pse.tile([128, 3, 512], f32, tag="e3")
                d3 = psd.tile([128, 3, 512], f32, tag="d3")

                if p < len(_PASSES) - 1:
                    # chunk-pair-outer matmuls (DoubleRow: contract 256).
                    for ch2 in range(NP2):
                        for j in range(nbp):
                            nc.tensor.matmul(
                                e3[:, j, 0:FW],
                                cpass[:, 0, ch2, :, 128 * j:128 * (j + 1)],
                                we_t[:, ch2, :, 0:FW],
                                start=(ch2 == 0), stop=(ch2 == NP2 - 1),
                                perf_mode=DR)
                    # previous pass' colsum matmuls (inv is ready by now)
                    if pending:
                        emit_cs(pending)
                        pending = []
                    for ch2 in range(NP2):
                        for j in range(nbp):
                            nc.tensor.matmul(
                                d3[:, j, 0:FW],
                                cpass[:, 1, ch2, :, 128 * j:128 * (j + 1)],
                                wd_t[:, ch2, :, 0:FW],
                                start=(ch2 == 0), stop=(ch2 == NP2 - 1),
                                perf_mode=DR)

                    # post-chain: sq = PSUM^2; p = C^2+S^2; rs = sum p;
                    # q = sum p^2; inv = 1/rs
                    sq3 = sqp.tile([128, nbp, 2, 2, B2], f32, tag="sq")
                    p3 = ppool.tile([128, nbp, 2, B2], bf16, tag="p")
                    nc.scalar.activation(
                        out=sq3[:, :, 0], func=AF.Square,
                        in_=e3[:, 0:nbp, 0:FW].rearrange(
                            "p j (c k) -> p j c k", c=2))
                    nc.scalar.activation(
                        out=sq3[:, :, 1], func=AF.Square,
                        in_=d3[:, 0:nbp, 0:FW].rearrange(
                            "p j (c k) -> p j c k", c=2))
                    with nc.allow_low_precision(reason="bf16 validated"):
                        for j, blk in enumerate(blks):
                            nc.vector.tensor_add(p3[:, j], sq3[:, j, :, 0],
                                                 sq3[:, j, :, 1])
                        for j, blk in enumerate(blks):
                            nc.vector.tensor_reduce(
                                out=rq_t[:, blk:blk + 1], in_=p3[:, j],
                                axis=mybir.AxisListType.XY, op=ALU.add)
                    inv3 = sm.tile([128, nbp], bf16, tag="inv",
                                   name=f"inv{p}")
                    with nc.allow_low_precision(reason="bf16 validated"):
                        nc.vector.reciprocal(inv3, rq_t[:, b0:b0 + nbp])
                    for j, blk in enumerate(blks):
                        psq = qpool.tile([128, FW], bf16, tag="psq",
                                         name=f"psq{blk}")
                        nc.scalar.activation(
                            out=psq, in_=p3[:, j], func=AF.Square,
                            accum_out=rq_t[:, NB + blk:NB + blk + 1])
                        pending.append((inv3[:, j:j + 1], p3, j, blk))
                else:
                    # final pass: block-outer matmuls + per-block post-chain
                    # so earlier blocks' stats chains overlap later blocks'
                    # matmuls (short kernel tail).  rs comes from the Square
                    # accumulators (rs_e + rs_d) to skip the 478-col reduce.
                    sq3 = sqp.tile([128, nbp, 2, 2, B2], f32, tag="sq")
                    p3 = ppool.tile([128, nbp, 2, B2], bf16, tag="p")
                    for j, blk in enumerate(blks):
                        for ch2 in range(NP2):
                            nc.tensor.matmul(
                                e3[:, j, 0:FW],
                                cpass[:, 0, ch2, :, 128 * j:128 * (j + 1)],
                                we_t[:, ch2, :, 0:FW],
                                start=(ch2 == 0), stop=(ch2 == NP2 - 1),
                                perf_mode=DR)
                        for ch2 in range(NP2):
                            nc.tensor.matmul(
                                d3[:, j, 0:FW],
                                cpass[:, 1, ch2, :, 128 * j:128 * (j + 1)],
                                wd_t[:, ch2, :, 0:FW],
                                start=(ch2 == 0), stop=(ch2 == NP2 - 1),
                                perf_mode=DR)
                        if pending:
                            emit_cs(pending)
                            pending = []
                        rse = sm.tile([128, 2], f32, tag="rse",
                                      name=f"rse{blk}")
                        nc.scalar.activation(
                            out=sq3[:, j, 0], func=AF.Square,
                            in_=e3[:, j, 0:FW].rearrange("p (c k) -> p c k",
                                                         c=2),
                            accum_out=rse[:, 0:1])
                        nc.scalar.activation(
                            out=sq3[:, j, 1], func=AF.Square,
                            in_=d3[:, j, 0:FW].rearrange("p (c k) -> p c k",
                                                         c=2),
                            accum_out=rse[:, 1:2])
                        inv1 = sm.tile([128, 1], bf16, tag="inv1",
                                       name=f"inv1_{blk}")
                        with nc.allow_low_precision(reason="bf16 validated"):
                            # e-half of p assembles during the d matmuls;
                            # only the d-half add sits on the kernel tail
                            nc.vector.tensor_add(p3[:, j, 0],
                                                 sq3[:, j, 0, 0],
                                                 sq3[:, j, 0, 1])
                            nc.gpsimd.tensor_add(p3[:, j, 1],
                                                 sq3[:, j, 1, 0],
                                                 sq3[:, j, 1, 1])
                            nc.vector.tensor_add(rq_t[:, blk:blk + 1],
                                                 rse[:, 0:1], rse[:, 1:2])
                            nc.vector.reciprocal(inv1,
                                                 rq_t[:, blk:blk + 1])
                        psq = qpool.tile([128, FW], bf16, tag="psq",
                                         name=f"psq{blk}")
                        nc.scalar.activation(
                            out=psq, in_=p3[:, j], func=AF.Square,
                            accum_out=rq_t[:, NB + blk:NB + blk + 1])
                        pending.append((inv1, p3, j, blk))

            emit_cs(pending)

            cs_sb = outp.tile([1, FW], f32)
            nc.vector.tensor_copy(cs_sb, cs_psum[:, 0:FW])
            nc.gpsimd.dma_start(out=out_rq[:], in_=rq_t)
            nc.sync.dma_start(out=out_cs[:], in_=cs_sb)

    nc.compile()
    _NC = nc
    return nc


def _host_prepare(model_output, GT_sig, offsets_st, offsets_t):
    """Build per-core in_maps: gather crops, e/d split, fp8 layout."""
    from numpy.lib.stride_tricks import sliding_window_view
    f8 = ml_dtypes.float8_e4m3
    w_e, w_d = _band_tables()
    mats = []   # 4 matrices' crop rows [2048, 4096] float32
    for b in range(B):
        offs = np.asarray(offsets_st[b], dtype=np.int64).reshape(-1)
        ch_idx = np.repeat(np.arange(C), K_CROPS)
        base = np.asarray(model_output[b], dtype=np.float32)
        win = sliding_window_view(base, L, axis=-1)  # [C, T-L+1, L]
        mats.append(win[ch_idx, offs])               # [2048, L]
    for b in range(B):
        offs = np.asarray(offsets_t[b], dtype=np.int64).reshape(-1)
        win = sliding_window_view(np.asarray(GT_sig[b], dtype=np.float32), L)
        mats.append(win[offs])

    in_maps = []
    for m in range(4):
        cr = mats[m]
        e = cr[:, :HALF] + cr[:, HALF:]
        d = cr[:, :HALF] - cr[:, HALF:]
        # [rows, 2, 2048] -> [rows, 2, pair, ktile, 128] -> [128, 2, pair, ktile, rows]
        x = np.stack([e, d], axis=1).reshape(-1, 2, NP2, 2, 128)
        for h in range(2):
            sl = x[h * ROWS_PER_CORE:(h + 1) * ROWS_PER_CORE]
            xr = np.ascontiguousarray(
                sl.transpose(4, 1, 2, 3, 0)).astype(f8)  # [128,2,8,2,1024]
            im = {"w_e": w_e, "w_d": w_d}
            c0 = 0
            for p, blks in enumerate(_PASSES):
                c1 = c0 + 128 * len(blks)
                im[f"crops_p{p}"] = np.ascontiguousarray(xr[..., c0:c1])
                c0 = c1
            in_maps.append(im)
    return in_maps


def _combine(results, label_flag):
    """results: list of 8 dicts with out_cs [1,478], out_rq [128,16]."""
    cs = np.zeros((4, FW), dtype=np.float64)
    ssq = np.zeros(4, dtype=np.float64)
    for m in range(4):
        for h in range(2):
            r = results[2 * m + h]
            cs[m] += np.asarray(r["out_cs"], dtype=np.float64)[0]
            rq = np.asarray(r["out_rq"], dtype=np.float64)
            rs = rq[:, 0:NB]
            q = rq[:, NB:2 * NB]
            ssq[m] += float(np.sum(q / (rs * rs)))

    N = float(N_ROWS)

    def cmp_excl(a):
        return (2.0 * N * ssq[a] - 2.0 * np.dot(cs[a], cs[a])) / F / (N * (N - 1.0))

    def cmp_full(a, b):
        return (N * ssq[a] + N * ssq[b] - 2.0 * np.dot(cs[a], cs[b])) / F / (N * N)

    lf = np.asarray(label_flag, dtype=np.float64).reshape(-1)
    lf_sum = lf[0] + lf[1]
    denom = 1.0 if lf_sum == 0 else lf_sum
    pos_loss = (cmp_excl(0) + cmp_excl(1)) / 2.0
    neg_loss = -cmp_full(0, 1)
    pos_GT = (lf[0] * cmp_full(0, 2) + lf[1] * cmp_full(1, 3)) / denom
    neg_GT = -(lf[0] * cmp_full(1, 2) + lf[1] * cmp_full(0, 3)) / denom
    if lf_sum == 0:
        pos_GT = 0.0
        neg_GT = 0.0
    loss = pos_loss + neg_loss + pos_GT + neg_GT
    return (np.float32(loss), np.float32(pos_loss), np.float32(neg_loss),
            np.float32(pos_GT), np.float32(neg_GT))


def run(inputs, trace=False):
    """Returns (outputs_tuple, BassKernelResults)."""
    from concourse import bass_utils
    nc = _build_module()
    in_maps = _host_prepare(
        inputs["model_output"], inputs["GT_sig"],
        inputs["offsets_st"], inputs["offsets_t"])
    res = bass_utils.run_bass_kernel_spmd(
        nc, in_maps, core_ids=list(range(N_CORES)), trace=trace)
    outs = _combine(res.results, inputs["label_flag"])
    return outs, res


def kernel(**inputs):
    outs, _ = run(inputs)
    return outs
